# revision 2
# baseline (speedup 1.0000x reference)
"""GQA kernel for trn2: B=2, L=2048, D=2048, Hq=32, Hkv=8, dh=64.

Sharding: 1 KV head (= 4 contiguous Q heads) per core; Wq/Wk/Wv
column-sharded by head, Wo row-sharded.

I/O strategy (the wall-clock bottleneck is the ~25 MB/s tunneled
host<->device link, not device compute):
  - host sends each core only a [D, 512] column block of xT (2 MB bf16);
    an on-device AllGather over NeuronLink rebuilds the full xT.
  - each core's Wo-partial output is ReduceScattered on device (f32),
    cast to bf16, and each core returns only its 512-row shard.
    Host just concatenates the 8 shards.

Layout trick: x is transposed on the host (xT: [D, B*L]) so every
on-device matmul has its contraction dim on partitions without any
on-device transposes:
  Q^T[dq, l]  = (Wq_tile).T @ xT        (lhsT=Wq, rhs=xT)
  K^T[dh, l]  = (Wk_tile).T @ xT
  V[l, dh]    = (xT_tile).T @ Wv        (lhsT=xT, rhs=Wv)
  S^T[k, q]   = (K^T_tile).T @ Q^T      (lhsT=K^T, rhs=Q^T)   contract dh=64
  E           = exp(S^T / 8)            (ScalarE, PSUM->SBUF)
  U[0:65, q]  = [V|1].T @ E             (lhsT=V_aug, rhs=E)   contract Lk
                row 64 of U = softmax denominator (ones column trick)
  attnT       = U[:64] * bcast(1/U[64]) (DVE recip + K=1 matmul bcast + mul)
  po[l, :]   += (attnT_tile).T @ Wo     (lhsT=attnT, rhs=Wo)
"""

import ml_dtypes
import numpy as np

import concourse.bass as bass
import concourse.bacc as bacc
import concourse.mybir as mybir
from concourse.tile import TileContext, add_dep_helper
from concourse.bass_utils import run_bass_kernel_spmd

B, L, D = 2, 2048, 2048
HQ, HKV, DH = 32, 8, 64
GQ = HQ // HKV            # 4 q heads per core
DQ = GQ * DH              # 256
BL = B * L                # 4096
P = 128
NB = 512                  # free-dim block
KD = D // P               # 16 contraction tiles over D
LT = L // P               # 16 Lk tiles per batch
NBLK = L // NB            # 4 Lq blocks per batch
NC = 8                    # cores
SH = BL // NC             # 512 output rows per core after reduce-scatter
SCALE = 1.0 / 8.0         # 1/sqrt(dh)

F32 = mybir.dt.float32
BF16 = mybir.dt.bfloat16
AF = mybir.ActivationFunctionType
GROUPS = [list(range(NC))]

_CACHED = {}


def _pe_sync(nc, producers, reason):
    # Hoist multi-source waits onto a PE nop: the self-loading f32r matmul
    # (S3_LW) can only carry a single sync wait in walrus codegen.
    if not producers:
        return
    nop = nc.tensor.nop(nofuse=True, hint="sponge")
    for p in producers:
        add_dep_helper(nop.ins, p.ins, reason=reason)


def build_nc():
    nc = bacc.Bacc()
    xcol = nc.declare_dram_parameter("xcol", [D, NB], BF16, isOutput=False)
    wq = nc.declare_dram_parameter("wq", [D, DQ], BF16, isOutput=False)
    wk = nc.declare_dram_parameter("wk", [D, 2 * DH], BF16, isOutput=False)
    wv = nc.declare_dram_parameter("wv", [D, DH], BF16, isOutput=False)
    wo = nc.declare_dram_parameter("wo", [DQ, D], BF16, isOutput=False)
    out = nc.declare_dram_parameter("out", [SH, D], BF16, isOutput=True)

    with TileContext(nc) as tc:
        with (
            tc.tile_pool(name="dpool", bufs=1, space="DRAM") as dpool,
            tc.tile_pool(name="wpool", bufs=1) as wpool,
            tc.tile_pool(name="xpool", bufs=3) as xpool,
            tc.tile_pool(name="qtpool", bufs=3) as qtpool,
            tc.tile_pool(name="ktpool", bufs=2) as ktpool,
            tc.tile_pool(name="vpool", bufs=34) as vpool,
            tc.tile_pool(name="epool", bufs=20) as epool,
            tc.tile_pool(name="atpool", bufs=2) as atpool,
            tc.tile_pool(name="opool", bufs=3) as opool,
            tc.tile_pool(name="bcpool", bufs=2) as bcpool,
            tc.tile_pool(name="rpool", bufs=4) as rpool,
            tc.tile_pool(name="psA", bufs=2, space="PSUM") as psA,
            tc.tile_pool(name="psS", bufs=4, space="PSUM") as psS,
            tc.tile_pool(name="psU", bufs=2, space="PSUM") as psU,
        ):
            # ---- DRAM staging for collectives ----
            xin = dpool.tile([D, NB], BF16, tag="xin")
            xg = dpool.tile([NC * D, NB], BF16, tag="xg")
            po = dpool.tile([BL, D], F32, tag="po")
            ro = dpool.tile([SH, D], F32, tag="ro")

            # AllGather the 8 xT column blocks: xg rows [g*D:(g+1)*D] end up
            # holding xT[:, g*NB:(g+1)*NB] (replica g's block).
            nc.gpsimd.dma_start(xin[:], xcol[:])
            nc.gpsimd.collective_compute(
                "AllGather",
                mybir.AluOpType.bypass,
                replica_groups=GROUPS,
                ins=[xin[:].opt()],
                outs=[xg[:].opt()],
            )
            xg_v = xg.rearrange("(g k p) n -> p g k n", p=P, k=KD)

            # ---- persistent weights ----
            wdmas = []
            wq_sb = wpool.tile([P, KD, DQ], BF16, tag="wq")
            wdmas.append(nc.sync.dma_start(out=wq_sb, in_=wq.rearrange("(k p) m -> p k m", p=P)))
            wk_sb = wpool.tile([P, KD, 2 * DH], BF16, tag="wk")
            wdmas.append(nc.sync.dma_start(out=wk_sb, in_=wk.rearrange("(k p) m -> p k m", p=P)))
            wv_sb = wpool.tile([P, KD, DH], BF16, tag="wv")
            wdmas.append(nc.sync.dma_start(out=wv_sb, in_=wv.rearrange("(k p) m -> p k m", p=P)))
            wo_sb = [wpool.tile([P, D], BF16, tag=f"wo{t}", name=f"wo_sb{t}") for t in range(2)]
            for t in range(2):
                wdmas.append(nc.sync.dma_start(out=wo_sb[t], in_=wo[t * P : (t + 1) * P, :]))
            ones_sb = wpool.tile([1, DH], BF16, tag="ones")
            nc.vector.memset(ones_sb, 1.0)

            for b in range(B):
                # ---------- phase A: projections for batch b ----------
                qt_sb = [qtpool.tile([P, L], BF16, tag="qt", name=f"qt_sb{t}") for t in range(2)]
                kt_sb = ktpool.tile([P, L], BF16, tag="kt")
                v_sb = [vpool.tile([P, DH + 1], BF16, tag="v", name=f"v_sb{k}") for k in range(LT)]
                acopies = []

                for c in range(NBLK):
                    gblk = b * NBLK + c  # global 512-col block of xT
                    xt_all = xpool.tile([P, KD, NB], BF16, tag="xt")
                    xdma = nc.sync.dma_start(out=xt_all, in_=xg_v[:, gblk, :, :])

                    # Q^T (two 128-row dq tiles)
                    for t in range(2):
                        q_ps = psA.tile([P, NB], F32, tag="acc")
                        for k in range(KD):
                            nc.tensor.matmul(
                                q_ps,
                                lhsT=wq_sb[:, k, t * P : (t + 1) * P],
                                rhs=xt_all[:, k, :],
                                start=(k == 0),
                                stop=(k == KD - 1),
                            )
                        acopies.append(nc.vector.tensor_copy(
                            qt_sb[t][:, c * NB : (c + 1) * NB], q_ps
                        ))
                    # K^T
                    k_ps = psA.tile([P, NB], F32, tag="acc")
                    for k in range(KD):
                        nc.tensor.matmul(
                            k_ps,
                            lhsT=wk_sb[:, k, :],
                            rhs=xt_all[:, k, :],
                            start=(k == 0),
                            stop=(k == KD - 1),
                        )
                    acopies.append(nc.vector.tensor_copy(kt_sb[:, c * NB : (c + 1) * NB], k_ps))
                    # V (natural, Lk-major) + ones column
                    for j in range(NB // P):
                        lk = c * (NB // P) + j
                        v_ps = psA.tile([P, DH], F32, tag="acc")
                        for k in range(KD):
                            nc.tensor.matmul(
                                v_ps,
                                lhsT=xt_all[:, k, j * P : (j + 1) * P],
                                rhs=wv_sb[:, k, :],
                                start=(k == 0),
                                stop=(k == KD - 1),
                            )
                        acopies.append(nc.vector.tensor_copy(v_sb[lk][:, :DH], v_ps))
                        acopies.append(nc.vector.memset(v_sb[lk][:, DH : DH + 1], 1.0))

                # ---------- phases B+C per Lq block ----------
                for c in range(NBLK):
                    at_sb = [atpool.tile([P, NB], BF16, tag="at", name=f"at_sb{t}") for t in range(2)]
                    at_producers = []
                    for g in range(GQ):
                        qg = qt_sb[g // 2][
                            (g % 2) * DH : (g % 2) * DH + DH, c * NB : (c + 1) * NB
                        ]
                        # S^T tiles + exp; interleave PV to keep PE/ACT in step
                        e_sb = []
                        u_ps = psU.tile([P, NB], F32, tag="u")

                        h0 = (g % 2) * DH

                        def qk_step(k):
                            sT = psS.tile([P, NB], F32, tag="sT")
                            nc.tensor.matmul(
                                sT,
                                lhsT=kt_sb[h0 : h0 + DH, k * P : (k + 1) * P],
                                rhs=qg,
                                start=True,
                                stop=True,
                            )
                            e = epool.tile([P, NB], BF16, tag="e")
                            nc.scalar.activation(e, sT, AF.Exp, scale=SCALE)
                            e_sb.append(e)

                        def pv_step(k):
                            nc.tensor.matmul(
                                u_ps[: DH + 1, :],
                                lhsT=v_sb[k][:, :],
                                rhs=e_sb[k],
                                start=(k == 0),
                                stop=(k == LT - 1),
                            )

                        for k in range(4):
                            qk_step(k)
                        for k in range(4, LT):
                            qk_step(k)
                            pv_step(k - 4)
                        for k in range(LT - 4, LT):
                            pv_step(k)

                        # normalize: attnT = U[:64] * bcast(1 / U[64])
                        recip = rpool.tile([1, NB], BF16, tag="r")
                        with nc.allow_low_precision(reason="f32r is fp32-width"):
                            nc.vector.reciprocal(recip, u_ps[DH : DH + 1, :])
                        bc_ps = psS.tile([DH, NB], F32, tag="sT")
                        nc.tensor.matmul(
                            bc_ps, lhsT=ones_sb, rhs=recip, start=True, stop=True
                        )
                        bc_sb = bcpool.tile([DH, NB], F32, tag="bc")
                        nc.vector.tensor_copy(bc_sb, bc_ps)
                        if g % 2 == 0:
                            at_producers.append(nc.vector.tensor_mul(
                                at_sb[g // 2][:DH, :], u_ps[:DH, :], bc_sb
                            ))
                        else:
                            at_tmp = rpool.tile([DH, NB], BF16, tag="at_tmp")
                            nc.vector.tensor_mul(at_tmp, u_ps[:DH, :], bc_sb)
                            at_producers.append(nc.sync.dma_start(
                                out=at_sb[g // 2][DH : 2 * DH, :], in_=at_tmp
                            ))

                    # ---- phase C: O-projection for this Lq block ----
                    for lt in range(NB // P):
                        row0 = b * L + c * NB + lt * P
                        for nb in range(D // NB):
                            o_ps = psA.tile([P, NB], F32, tag="acc")
                            for t in range(2):
                                nc.tensor.matmul(
                                    o_ps,
                                    lhsT=at_sb[t][:, lt * P : (lt + 1) * P],
                                    rhs=wo_sb[t][:, nb * NB : (nb + 1) * NB],
                                    start=(t == 0),
                                    stop=(t == 1),
                                )
                            o_sb = opool.tile([P, NB], F32, tag="o")
                            nc.vector.tensor_copy(o_sb, o_ps)
                            nc.sync.dma_start(
                                out=po[row0 : row0 + P, nb * NB : (nb + 1) * NB],
                                in_=o_sb,
                            )

            # ---- reduce partials across cores; each core keeps 512 rows ----
            nc.gpsimd.collective_compute(
                "ReduceScatter",
                mybir.AluOpType.add,
                replica_groups=GROUPS,
                ins=[po[:].opt()],
                outs=[ro[:].opt()],
            )
            # f32 -> bf16 cast through SBUF, then to the external output
            for t in range(SH // P):
                r_sb = opool.tile([P, D], F32, tag="rcast")
                nc.sync.dma_start(out=r_sb, in_=ro[t * P : (t + 1) * P, :])
                rb_sb = opool.tile([P, D], BF16, tag="rcastb")
                nc.vector.tensor_copy(rb_sb, r_sb)
                nc.sync.dma_start(out=out[t * P : (t + 1) * P, :], in_=rb_sb)
    nc.compile()
    return nc


def kernel(x, Wq, Wk, Wv, Wo, trace=False):
    x = np.ascontiguousarray(np.asarray(x, dtype=np.float32))
    Wq = np.asarray(Wq, dtype=np.float32).astype(ml_dtypes.bfloat16)
    Wk = np.asarray(Wk, dtype=np.float32).astype(ml_dtypes.bfloat16)
    Wv = np.asarray(Wv, dtype=np.float32).astype(ml_dtypes.bfloat16)
    Wo = np.asarray(Wo, dtype=np.float32).astype(ml_dtypes.bfloat16)

    xT = np.ascontiguousarray(x.reshape(BL, D).T.astype(ml_dtypes.bfloat16))  # [D, BL]

    in_maps = []
    for i in range(NC):
        qs = slice(i * DQ, (i + 1) * DQ)
        ks = slice(i * DH, (i + 1) * DH)
        in_maps.append(
            {
                "xcol": np.ascontiguousarray(xT[:, i * NB : (i + 1) * NB]),
                "wq": np.ascontiguousarray(Wq[:, qs]),
                "wk": np.ascontiguousarray(np.concatenate([Wk[:, ks], Wk[:, ks]], axis=1)),
                "wv": np.ascontiguousarray(Wv[:, ks]),
                "wo": np.ascontiguousarray(Wo[qs, :]),
            }
        )

    if "nc" not in _CACHED:
        _CACHED["nc"] = build_nc()
    nc = _CACHED["nc"]

    res = run_bass_kernel_spmd(nc, in_maps, list(range(NC)), trace=trace)
    acc = np.concatenate([r["out"] for r in res.results], axis=0).astype(np.float32)
    if trace:
        kernel.last_exec_time_ns = res.exec_time_ns
        kernel.last_results = res
    return acc.reshape(B, L, D)


# revision 7
# speedup vs baseline: 1.0097x; 1.0097x over previous
"""GQA kernel for trn2: B=2, L=2048, D=2048, Hq=32, Hkv=8, dh=64.

Sharding: 1 KV head (= 4 contiguous Q heads) per core; Wq/Wk/Wv
column-sharded by head, Wo row-sharded.

I/O strategy (the wall-clock bottleneck is the ~25 MB/s tunneled
host<->device link, not device compute):
  - host sends each core only a [D, 512] column block of xT (2 MB bf16);
    an on-device AllGather over NeuronLink rebuilds the full xT.
  - each core's Wo-partial output is ReduceScattered on device (f32),
    cast to bf16, and each core returns only its 512-row shard.
    Host just concatenates the 8 shards.

Layout trick: x is transposed on the host (xT: [D, B*L]) so every
on-device matmul has its contraction dim on partitions without any
on-device transposes:
  Q^T[dq, l]  = (Wq_tile).T @ xT        (lhsT=Wq, rhs=xT)
  K^T[dh, l]  = (Wk_tile).T @ xT
  V[l, dh]    = (xT_tile).T @ Wv        (lhsT=xT, rhs=Wv)
  S^T[k, q]   = (K^T_tile).T @ Q^T      (lhsT=K^T, rhs=Q^T)   contract dh=64
  E           = exp(S^T / 8)            (ScalarE, PSUM->SBUF)
  U[0:65, q]  = [V|1].T @ E             (lhsT=V_aug, rhs=E)   contract Lk
                row 64 of U = softmax denominator (ones column trick)
  attnT       = U[:64] * bcast(1/U[64]) (DVE recip + K=1 matmul bcast + mul)
  po[l, :]   += (attnT_tile).T @ Wo     (lhsT=attnT, rhs=Wo)
"""

import ml_dtypes
import numpy as np

try:  # persistent XLA compile cache: skips ~0.3s of per-call recompilation
    import jax

    jax.config.update("jax_compilation_cache_dir", "/tmp/jax_comp_cache")
    jax.config.update("jax_persistent_cache_min_compile_time_secs", 0.0)
    jax.config.update("jax_persistent_cache_min_entry_size_bytes", 0)
except Exception:
    pass

import concourse.bass as bass
import concourse.bacc as bacc
import concourse.mybir as mybir
from concourse.tile import TileContext, add_dep_helper
from concourse.bass_utils import run_bass_kernel_spmd

B, L, D = 2, 2048, 2048
HQ, HKV, DH = 32, 8, 64
GQ = HQ // HKV            # 4 q heads per core
DQ = GQ * DH              # 256
BL = B * L                # 4096
P = 128
NB = 512                  # free-dim block
KD = D // P               # 16 contraction tiles over D
LT = L // P               # 16 Lk tiles per batch
NBLK = L // NB            # 4 Lq blocks per batch
NC = 8                    # cores
SH = BL // NC             # 512 output rows per core after reduce-scatter
SCALE = 1.0 / 8.0         # 1/sqrt(dh)

F32 = mybir.dt.float32
BF16 = mybir.dt.bfloat16
AF = mybir.ActivationFunctionType
GROUPS = [list(range(NC))]

_CACHED = {}


def _pe_sync(nc, producers, reason):
    # Hoist multi-source waits onto a PE nop: the self-loading f32r matmul
    # (S3_LW) can only carry a single sync wait in walrus codegen.
    if not producers:
        return
    nop = nc.tensor.nop(nofuse=True, hint="sponge")
    for p in producers:
        add_dep_helper(nop.ins, p.ins, reason=reason)


def build_nc():
    nc = bacc.Bacc()
    xrow = nc.declare_dram_parameter("xrow", [SH, D], BF16, isOutput=False)
    wq = nc.declare_dram_parameter("wq", [D, DQ], BF16, isOutput=False)
    wk = nc.declare_dram_parameter("wk", [D, DH], BF16, isOutput=False)
    wv = nc.declare_dram_parameter("wv", [D, DH], BF16, isOutput=False)
    wo = nc.declare_dram_parameter("wo", [DQ, D], BF16, isOutput=False)
    out = nc.declare_dram_parameter("out", [SH, D], BF16, isOutput=True)

    with TileContext(nc) as tc:
        with (
            tc.tile_pool(name="dpool", bufs=1, space="DRAM") as dpool,
            tc.tile_pool(name="wpool", bufs=1) as wpool,
            tc.tile_pool(name="xpool", bufs=3) as xpool,
            tc.tile_pool(name="qtpool", bufs=3) as qtpool,
            tc.tile_pool(name="ktpool", bufs=2) as ktpool,
            tc.tile_pool(name="vpool", bufs=34) as vpool,
            tc.tile_pool(name="epool", bufs=20) as epool,
            tc.tile_pool(name="atpool", bufs=2) as atpool,
            tc.tile_pool(name="opool", bufs=3) as opool,
            tc.tile_pool(name="bcpool", bufs=2) as bcpool,
            tc.tile_pool(name="rpool", bufs=4) as rpool,
            tc.tile_pool(name="psA", bufs=2, space="PSUM") as psA,
            tc.tile_pool(name="psS", bufs=4, space="PSUM") as psS,
            tc.tile_pool(name="psU", bufs=2, space="PSUM") as psU,
        ):
            # ---- DRAM staging for collectives ----
            xin = dpool.tile([D, NB], BF16, tag="xin")
            xg = dpool.tile([NC * D, NB], BF16, tag="xg")
            po = dpool.tile([BL, D], F32, tag="po")
            ro = dpool.tile([SH, D], F32, tag="ro")

            # On-device transpose of this core's 512 rows of x (bf16 XBAR
            # transpose DMA, DRAM->SBUF), then SBUF->DRAM so the AllGather can
            # read it: xg rows [g*D:(g+1)*D] end up holding xT[:, g*NB:(g+1)*NB]
            # (replica g's block).
            xts = xpool.tile([P, KD, NB], BF16, tag="xts")
            for dt in range(KD):
                nc.sync.dma_start(
                    out=xts[:, dt, :], in_=xrow[:, dt * P : (dt + 1) * P], transpose=True
                )
            nc.sync.dma_start(out=xin.rearrange("(k p) n -> p k n", p=P), in_=xts)
            nc.gpsimd.collective_compute(
                "AllGather",
                mybir.AluOpType.bypass,
                replica_groups=GROUPS,
                ins=[xin[:].opt()],
                outs=[xg[:].opt()],
            )
            xg_v = xg.rearrange("(g k p) n -> p g k n", p=P, k=KD)

            # ---- persistent weights ----
            wdmas = []
            wq_sb = wpool.tile([P, KD, DQ], BF16, tag="wq")
            wdmas.append(nc.sync.dma_start(out=wq_sb, in_=wq.rearrange("(k p) m -> p k m", p=P)))
            # K weights are used from both partition halves of kt_sb; load the
            # single [D, DH] input into both column halves instead of shipping
            # a duplicated [D, 2*DH] tensor over the host link.
            wk_sb = wpool.tile([P, KD, 2 * DH], BF16, tag="wk")
            wk_v = wk.rearrange("(k p) m -> p k m", p=P)
            wdmas.append(nc.sync.dma_start(out=wk_sb[:, :, 0:DH], in_=wk_v))
            wdmas.append(nc.sync.dma_start(out=wk_sb[:, :, DH : 2 * DH], in_=wk_v))
            wv_sb = wpool.tile([P, KD, DH], BF16, tag="wv")
            wdmas.append(nc.sync.dma_start(out=wv_sb, in_=wv.rearrange("(k p) m -> p k m", p=P)))
            wo_sb = [wpool.tile([P, D], BF16, tag=f"wo{t}", name=f"wo_sb{t}") for t in range(2)]
            for t in range(2):
                wdmas.append(nc.sync.dma_start(out=wo_sb[t], in_=wo[t * P : (t + 1) * P, :]))
            ones_sb = wpool.tile([1, DH], BF16, tag="ones")
            nc.vector.memset(ones_sb, 1.0)

            for b in range(B):
                # ---------- phase A: projections for batch b ----------
                qt_sb = [qtpool.tile([P, L], BF16, tag="qt", name=f"qt_sb{t}") for t in range(2)]
                kt_sb = ktpool.tile([P, L], BF16, tag="kt")
                v_sb = [vpool.tile([P, DH + 1], BF16, tag="v", name=f"v_sb{k}") for k in range(LT)]
                acopies = []

                for c in range(NBLK):
                    gblk = b * NBLK + c  # global 512-col block of xT
                    xt_all = xpool.tile([P, KD, NB], BF16, tag="xt")
                    xdma = nc.sync.dma_start(out=xt_all, in_=xg_v[:, gblk, :, :])

                    # Q^T (two 128-row dq tiles)
                    for t in range(2):
                        q_ps = psA.tile([P, NB], F32, tag="acc")
                        for k in range(KD):
                            nc.tensor.matmul(
                                q_ps,
                                lhsT=wq_sb[:, k, t * P : (t + 1) * P],
                                rhs=xt_all[:, k, :],
                                start=(k == 0),
                                stop=(k == KD - 1),
                            )
                        acopies.append(nc.vector.tensor_copy(
                            qt_sb[t][:, c * NB : (c + 1) * NB], q_ps
                        ))
                    # K^T
                    k_ps = psA.tile([P, NB], F32, tag="acc")
                    for k in range(KD):
                        nc.tensor.matmul(
                            k_ps,
                            lhsT=wk_sb[:, k, :],
                            rhs=xt_all[:, k, :],
                            start=(k == 0),
                            stop=(k == KD - 1),
                        )
                    acopies.append(nc.vector.tensor_copy(kt_sb[:, c * NB : (c + 1) * NB], k_ps))
                    # V (natural, Lk-major) + ones column
                    for j in range(NB // P):
                        lk = c * (NB // P) + j
                        v_ps = psA.tile([P, DH], F32, tag="acc")
                        for k in range(KD):
                            nc.tensor.matmul(
                                v_ps,
                                lhsT=xt_all[:, k, j * P : (j + 1) * P],
                                rhs=wv_sb[:, k, :],
                                start=(k == 0),
                                stop=(k == KD - 1),
                            )
                        acopies.append(nc.vector.tensor_copy(v_sb[lk][:, :DH], v_ps))
                        acopies.append(nc.vector.memset(v_sb[lk][:, DH : DH + 1], 1.0))

                # ---------- phases B+C per Lq block ----------
                for c in range(NBLK):
                    at_sb = [atpool.tile([P, NB], BF16, tag="at", name=f"at_sb{t}") for t in range(2)]
                    at_producers = []
                    for g in range(GQ):
                        qg = qt_sb[g // 2][
                            (g % 2) * DH : (g % 2) * DH + DH, c * NB : (c + 1) * NB
                        ]
                        # S^T tiles + exp; interleave PV to keep PE/ACT in step
                        e_sb = []
                        u_ps = psU.tile([P, NB], F32, tag="u")

                        h0 = (g % 2) * DH

                        def qk_step(k):
                            sT = psS.tile([P, NB], F32, tag="sT")
                            nc.tensor.matmul(
                                sT,
                                lhsT=kt_sb[h0 : h0 + DH, k * P : (k + 1) * P],
                                rhs=qg,
                                start=True,
                                stop=True,
                            )
                            e = epool.tile([P, NB], BF16, tag="e")
                            nc.scalar.activation(e, sT, AF.Exp, scale=SCALE)
                            e_sb.append(e)

                        def pv_step(k):
                            nc.tensor.matmul(
                                u_ps[: DH + 1, :],
                                lhsT=v_sb[k][:, :],
                                rhs=e_sb[k],
                                start=(k == 0),
                                stop=(k == LT - 1),
                            )

                        for k in range(4):
                            qk_step(k)
                        for k in range(4, LT):
                            qk_step(k)
                            pv_step(k - 4)
                        for k in range(LT - 4, LT):
                            pv_step(k)

                        # normalize: attnT = U[:64] * bcast(1 / U[64])
                        recip = rpool.tile([1, NB], BF16, tag="r")
                        with nc.allow_low_precision(reason="f32r is fp32-width"):
                            nc.vector.reciprocal(recip, u_ps[DH : DH + 1, :])
                        bc_ps = psS.tile([DH, NB], F32, tag="sT")
                        nc.tensor.matmul(
                            bc_ps, lhsT=ones_sb, rhs=recip, start=True, stop=True
                        )
                        bc_sb = bcpool.tile([DH, NB], F32, tag="bc")
                        nc.vector.tensor_copy(bc_sb, bc_ps)
                        if g % 2 == 0:
                            at_producers.append(nc.vector.tensor_mul(
                                at_sb[g // 2][:DH, :], u_ps[:DH, :], bc_sb
                            ))
                        else:
                            at_tmp = rpool.tile([DH, NB], BF16, tag="at_tmp")
                            nc.vector.tensor_mul(at_tmp, u_ps[:DH, :], bc_sb)
                            at_producers.append(nc.sync.dma_start(
                                out=at_sb[g // 2][DH : 2 * DH, :], in_=at_tmp
                            ))

                    # ---- phase C: O-projection for this Lq block ----
                    for lt in range(NB // P):
                        row0 = b * L + c * NB + lt * P
                        for nb in range(D // NB):
                            o_ps = psA.tile([P, NB], F32, tag="acc")
                            for t in range(2):
                                nc.tensor.matmul(
                                    o_ps,
                                    lhsT=at_sb[t][:, lt * P : (lt + 1) * P],
                                    rhs=wo_sb[t][:, nb * NB : (nb + 1) * NB],
                                    start=(t == 0),
                                    stop=(t == 1),
                                )
                            o_sb = opool.tile([P, NB], F32, tag="o")
                            nc.vector.tensor_copy(o_sb, o_ps)
                            nc.sync.dma_start(
                                out=po[row0 : row0 + P, nb * NB : (nb + 1) * NB],
                                in_=o_sb,
                            )

            # ---- reduce partials across cores; each core keeps 512 rows ----
            nc.gpsimd.collective_compute(
                "ReduceScatter",
                mybir.AluOpType.add,
                replica_groups=GROUPS,
                ins=[po[:].opt()],
                outs=[ro[:].opt()],
            )
            # f32 -> bf16 cast through SBUF (NB-wide chunks reuse opool's
            # existing [P, NB] slot size), then to the external output
            for t in range(SH // P):
                for nb in range(D // NB):
                    r_sb = opool.tile([P, NB], F32, tag="o")
                    nc.sync.dma_start(
                        out=r_sb, in_=ro[t * P : (t + 1) * P, nb * NB : (nb + 1) * NB]
                    )
                    rb_sb = opool.tile([P, NB], BF16, tag="o")
                    nc.vector.tensor_copy(rb_sb, r_sb)
                    nc.sync.dma_start(
                        out=out[t * P : (t + 1) * P, nb * NB : (nb + 1) * NB], in_=rb_sb
                    )
    nc.compile()
    return nc


def kernel(x, Wq, Wk, Wv, Wo, trace=False):
    # Only one contiguous bf16 cast on the host; sharding is pure views
    # (run_bass_kernel_spmd's concat does the single copy). x is transposed
    # on device (XBAR transpose DMA) and AllGathered over NeuronLink.
    xb = np.asarray(x, dtype=np.float32).reshape(BL, D).astype(ml_dtypes.bfloat16)
    Wq = np.asarray(Wq, dtype=np.float32).astype(ml_dtypes.bfloat16)
    Wk = np.asarray(Wk, dtype=np.float32).astype(ml_dtypes.bfloat16)
    Wv = np.asarray(Wv, dtype=np.float32).astype(ml_dtypes.bfloat16)
    Wo = np.asarray(Wo, dtype=np.float32).astype(ml_dtypes.bfloat16)

    in_maps = []
    for i in range(NC):
        qs = slice(i * DQ, (i + 1) * DQ)
        ks = slice(i * DH, (i + 1) * DH)
        in_maps.append(
            {
                "xrow": xb[i * SH : (i + 1) * SH],
                "wq": Wq[:, qs],
                "wk": Wk[:, ks],
                "wv": Wv[:, ks],
                "wo": Wo[qs, :],
            }
        )

    if "nc" not in _CACHED:
        _CACHED["nc"] = build_nc()
    nc = _CACHED["nc"]

    res = run_bass_kernel_spmd(nc, in_maps, list(range(NC)), trace=trace)
    acc = np.concatenate([r["out"] for r in res.results], axis=0).astype(np.float32)
    if trace:
        kernel.last_exec_time_ns = res.exec_time_ns
        kernel.last_results = res
    return acc.reshape(B, L, D)


# revision 40
# speedup vs baseline: 1.6834x; 1.6671x over previous
"""GQA kernel for trn2: B=2, L=2048, D=2048, Hq=32, Hkv=8, dh=64.

Sharding: 1 KV head (= 4 contiguous Q heads) per core; Wq/Wk/Wv
column-sharded by head, Wo row-sharded.

I/O strategy (the wall-clock bottleneck is the ~30-50 MB/s tunneled
host<->device link, not device compute):
  - all per-core inputs ship in ONE uint8 blob (one transfer stream):
    a [512, D] bf16 row-shard of x, bf16 Wq/Wk slices, and int8 Wo
    (per-column scales undone on the host after the reduce).
  - the x shard is transposed on device (XBAR transpose DMA) and
    AllGathered over NeuronLink to rebuild the full xT.
  - each core's Wo-partial is ReduceScattered on device (f32), then
    quantized to int8 with a per-row scale packed into the last 4 bytes
    of each output row; the host concatenates 8 shards and dequantizes.

Layout trick: every on-device matmul has its contraction dim on
partitions (xT: [D, B*L] built by the on-device transpose):
  Q^T[dq, l]  = (Wq_tile).T @ xT        (lhsT=Wq, rhs=xT)
  K^T[dh, l]  = (Wk_tile).T @ xT
  V[l, dh]    = (xT_tile).T @ Wv        (lhsT=xT, rhs=Wv)
  S^T[k, q]   = (K^T_tile).T @ Q^T      (lhsT=K^T, rhs=Q^T)   contract dh=64
  E           = exp(S^T / 8)            (ScalarE, PSUM->SBUF)
  U[0:65, q]  = [V|1].T @ E             (lhsT=V_aug, rhs=E)   contract Lk
                row 64 of U = softmax denominator (ones column trick)
  attnT       = U[:64] * bcast(1/U[64]) (DVE recip + K=1 matmul bcast + mul)
  po[l, :]   += (attnT_tile).T @ Wo     (lhsT=attnT, rhs=Wo)
"""

import ml_dtypes
import numpy as np

try:  # persistent XLA compile cache: skips ~0.3s of per-call recompilation
    import jax

    jax.config.update("jax_compilation_cache_dir", "/tmp/jax_comp_cache")
    jax.config.update("jax_persistent_cache_min_compile_time_secs", 0.0)
    jax.config.update("jax_persistent_cache_min_entry_size_bytes", 0)
except Exception:
    pass

import concourse.bass as bass
import concourse.bacc as bacc
import concourse.mybir as mybir
from concourse.tile import TileContext, add_dep_helper
from concourse.bass_utils import run_bass_kernel_spmd

B, L, D = 2, 2048, 2048
HQ, HKV, DH = 32, 8, 64
GQ = HQ // HKV            # 4 q heads per core
DQ = GQ * DH              # 256
BL = B * L                # 4096
P = 128
NB = 512                  # free-dim block
KD = D // P               # 16 contraction tiles over D
LT = L // P               # 16 Lk tiles per batch
NBLK = L // NB            # 4 Lq blocks per batch
NC = 8                    # cores
SH = BL // NC             # 512 output rows per core after reduce-scatter
SCALE = 1.0 / 8.0         # 1/sqrt(dh)

F32 = mybir.dt.float32
BF16 = mybir.dt.bfloat16
I8 = mybir.dt.int8
U8 = mybir.dt.uint8
AF = mybir.ActivationFunctionType
AL = mybir.AluOpType
AX = mybir.AxisListType
GROUPS = [list(range(NC))]
QMAX = 125.0  # int8 quant ceiling; < 127 absorbs DVE-reciprocal approx error

_CACHED = {}


def _pe_sync(nc, producers, reason):
    # Hoist multi-source waits onto a PE nop: the self-loading f32r matmul
    # (S3_LW) can only carry a single sync wait in walrus codegen.
    if not producers:
        return
    nop = nc.tensor.nop(nofuse=True, hint="sponge")
    for p in producers:
        add_dep_helper(nop.ins, p.ins, reason=reason)


OFF_X = 0
OFF_WQ = OFF_X + SH * D * 2
OFF_WK = OFF_WQ + D * DQ * 2
OFF_WV = OFF_WK + D * DH * 2
OFF_WO = OFF_WV + D * DH * 2
NBYTES = OFF_WO + DQ * D  # wo shipped as int8 (per-column scales dequant on host)


def build_nc():
    nc = bacc.Bacc()
    # All inputs packed into one uint8 blob (fewer host-link streams: each
    # separate array pays its own transfer-pipeline ramp on the tunneled
    # link). Slices are bitcast back to bf16 views below.
    blob = nc.declare_dram_parameter("blob", [NBYTES], U8, isOutput=False)
    xrow = blob[OFF_X:OFF_WQ].bitcast(BF16).rearrange("(l d) -> l d", d=D)
    wq = blob[OFF_WQ:OFF_WK].bitcast(BF16).rearrange("(k m) -> k m", m=DQ)
    wk = blob[OFF_WK:OFF_WV].bitcast(BF16).rearrange("(k m) -> k m", m=DH)
    wv = blob[OFF_WV:OFF_WO].bitcast(BF16).rearrange("(k m) -> k m", m=DH)
    # Wo arrives int8 (quantized per output column on the host; the shared
    # per-column scale is applied on the host after the reduce-scatter, so
    # the device only needs an exact int8 -> bf16 widening at load).
    wo = blob[OFF_WO:NBYTES].bitcast(I8).rearrange("(k m) -> k m", m=D)
    # int8 output + per-row quant scale (f32 bit-packed into the last 4
    # bytes of each row): halves the result + donated-zero-buffer bytes vs
    # bf16 and keeps everything in a single output stream.
    out_q = nc.declare_dram_parameter("out_q", [SH, D + 4], I8, isOutput=True)

    with TileContext(nc) as tc:
        with (
            tc.tile_pool(name="dpool", bufs=1, space="DRAM") as dpool,
            tc.tile_pool(name="wpool", bufs=1) as wpool,
            tc.tile_pool(name="xpool", bufs=3) as xpool,
            tc.tile_pool(name="qtpool", bufs=3) as qtpool,
            tc.tile_pool(name="ktpool", bufs=2) as ktpool,
            tc.tile_pool(name="vpool", bufs=34) as vpool,
            tc.tile_pool(name="epool", bufs=20) as epool,
            tc.tile_pool(name="atpool", bufs=2) as atpool,
            tc.tile_pool(name="opool", bufs=3) as opool,
            tc.tile_pool(name="bcpool", bufs=2) as bcpool,
            tc.tile_pool(name="rpool", bufs=4) as rpool,
            tc.tile_pool(name="psA", bufs=2, space="PSUM") as psA,
            tc.tile_pool(name="psS", bufs=4, space="PSUM") as psS,
            tc.tile_pool(name="psU", bufs=2, space="PSUM") as psU,
        ):
            # ---- DRAM staging for collectives ----
            xin = dpool.tile([D, NB], BF16, tag="xin")
            xg = dpool.tile([NC * D, NB], BF16, tag="xg")
            po = dpool.tile([BL, D], F32, tag="po")
            ro = dpool.tile([SH, D], F32, tag="ro")

            # On-device transpose of this core's 512 rows of x (bf16 XBAR
            # transpose DMA, DRAM->SBUF), then SBUF->DRAM so the AllGather can
            # read it: xg rows [g*D:(g+1)*D] end up holding xT[:, g*NB:(g+1)*NB]
            # (replica g's block).
            xts = xpool.tile([P, KD, NB], BF16, tag="xts")
            for dt in range(KD):
                nc.sync.dma_start(
                    out=xts[:, dt, :], in_=xrow[:, dt * P : (dt + 1) * P], transpose=True
                )
            nc.sync.dma_start(out=xin.rearrange("(k p) n -> p k n", p=P), in_=xts)
            nc.gpsimd.collective_compute(
                "AllGather",
                mybir.AluOpType.bypass,
                replica_groups=GROUPS,
                ins=[xin[:].opt()],
                outs=[xg[:].opt()],
            )
            xg_v = xg.rearrange("(g k p) n -> p g k n", p=P, k=KD)

            # ---- persistent weights ----
            wdmas = []
            wq_sb = wpool.tile([P, KD, DQ], BF16, tag="wq")
            wdmas.append(nc.sync.dma_start(out=wq_sb, in_=wq.rearrange("(k p) m -> p k m", p=P)))
            # K weights are used from both partition halves of kt_sb; load the
            # single [D, DH] input into both column halves instead of shipping
            # a duplicated [D, 2*DH] tensor over the host link.
            wk_sb = wpool.tile([P, KD, 2 * DH], BF16, tag="wk")
            wk_v = wk.rearrange("(k p) m -> p k m", p=P)
            wdmas.append(nc.sync.dma_start(out=wk_sb[:, :, 0:DH], in_=wk_v))
            wdmas.append(nc.sync.dma_start(out=wk_sb[:, :, DH : 2 * DH], in_=wk_v))
            wv_sb = wpool.tile([P, KD, DH], BF16, tag="wv")
            wdmas.append(nc.sync.dma_start(out=wv_sb, in_=wv.rearrange("(k p) m -> p k m", p=P)))
            wo_sb = [wpool.tile([P, D], BF16, tag=f"wo{t}", name=f"wo_sb{t}") for t in range(2)]
            for t in range(2):
                # [P, D] int8 staging tile = 2KB/partition, same as opool's slot
                wo_stage = opool.tile([P, D], I8, tag="o", name=f"wo_stage{t}")
                wdmas.append(nc.sync.dma_start(out=wo_stage, in_=wo[t * P : (t + 1) * P, :]))
                wdmas.append(nc.vector.tensor_copy(wo_sb[t], wo_stage))
            ones_sb = wpool.tile([1, DH], BF16, tag="ones")
            nc.vector.memset(ones_sb, 1.0)

            for b in range(B):
                # ---------- phase A: projections for batch b ----------
                qt_sb = [qtpool.tile([P, L], BF16, tag="qt", name=f"qt_sb{t}") for t in range(2)]
                kt_sb = ktpool.tile([P, L], BF16, tag="kt")
                v_sb = [vpool.tile([P, DH + 1], BF16, tag="v", name=f"v_sb{k}") for k in range(LT)]
                acopies = []

                for c in range(NBLK):
                    gblk = b * NBLK + c  # global 512-col block of xT
                    xt_all = xpool.tile([P, KD, NB], BF16, tag="xt")
                    xdma = nc.sync.dma_start(out=xt_all, in_=xg_v[:, gblk, :, :])

                    # Q^T (two 128-row dq tiles)
                    for t in range(2):
                        q_ps = psA.tile([P, NB], F32, tag="acc")
                        for k in range(KD):
                            nc.tensor.matmul(
                                q_ps,
                                lhsT=wq_sb[:, k, t * P : (t + 1) * P],
                                rhs=xt_all[:, k, :],
                                start=(k == 0),
                                stop=(k == KD - 1),
                            )
                        acopies.append(nc.vector.tensor_copy(
                            qt_sb[t][:, c * NB : (c + 1) * NB], q_ps
                        ))
                    # K^T
                    k_ps = psA.tile([P, NB], F32, tag="acc")
                    for k in range(KD):
                        nc.tensor.matmul(
                            k_ps,
                            lhsT=wk_sb[:, k, :],
                            rhs=xt_all[:, k, :],
                            start=(k == 0),
                            stop=(k == KD - 1),
                        )
                    acopies.append(nc.vector.tensor_copy(kt_sb[:, c * NB : (c + 1) * NB], k_ps))
                    # V (natural, Lk-major) + ones column
                    for j in range(NB // P):
                        lk = c * (NB // P) + j
                        v_ps = psA.tile([P, DH], F32, tag="acc")
                        for k in range(KD):
                            nc.tensor.matmul(
                                v_ps,
                                lhsT=xt_all[:, k, j * P : (j + 1) * P],
                                rhs=wv_sb[:, k, :],
                                start=(k == 0),
                                stop=(k == KD - 1),
                            )
                        acopies.append(nc.vector.tensor_copy(v_sb[lk][:, :DH], v_ps))
                        acopies.append(nc.vector.memset(v_sb[lk][:, DH : DH + 1], 1.0))

                # ---------- phases B+C per Lq block ----------
                for c in range(NBLK):
                    at_sb = [atpool.tile([P, NB], BF16, tag="at", name=f"at_sb{t}") for t in range(2)]
                    at_producers = []
                    for g in range(GQ):
                        qg = qt_sb[g // 2][
                            (g % 2) * DH : (g % 2) * DH + DH, c * NB : (c + 1) * NB
                        ]
                        # S^T tiles + exp; interleave PV to keep PE/ACT in step
                        e_sb = []
                        u_ps = psU.tile([P, NB], F32, tag="u")

                        h0 = (g % 2) * DH

                        def qk_step(k):
                            sT = psS.tile([P, NB], F32, tag="sT")
                            nc.tensor.matmul(
                                sT,
                                lhsT=kt_sb[h0 : h0 + DH, k * P : (k + 1) * P],
                                rhs=qg,
                                start=True,
                                stop=True,
                            )
                            e = epool.tile([P, NB], BF16, tag="e")
                            nc.scalar.activation(e, sT, AF.Exp, scale=SCALE)
                            e_sb.append(e)

                        def pv_step(k):
                            nc.tensor.matmul(
                                u_ps[: DH + 1, :],
                                lhsT=v_sb[k][:, :],
                                rhs=e_sb[k],
                                start=(k == 0),
                                stop=(k == LT - 1),
                            )

                        for k in range(4):
                            qk_step(k)
                        for k in range(4, LT):
                            qk_step(k)
                            pv_step(k - 4)
                        for k in range(LT - 4, LT):
                            pv_step(k)

                        # normalize: attnT = U[:64] * bcast(1 / U[64])
                        recip = rpool.tile([1, NB], BF16, tag="r")
                        with nc.allow_low_precision(reason="f32r is fp32-width"):
                            nc.vector.reciprocal(recip, u_ps[DH : DH + 1, :])
                        bc_ps = psS.tile([DH, NB], F32, tag="sT")
                        nc.tensor.matmul(
                            bc_ps, lhsT=ones_sb, rhs=recip, start=True, stop=True
                        )
                        bc_sb = bcpool.tile([DH, NB], F32, tag="bc")
                        nc.vector.tensor_copy(bc_sb, bc_ps)
                        if g % 2 == 0:
                            at_producers.append(nc.vector.tensor_mul(
                                at_sb[g // 2][:DH, :], u_ps[:DH, :], bc_sb
                            ))
                        else:
                            at_tmp = rpool.tile([DH, NB], BF16, tag="at_tmp")
                            nc.vector.tensor_mul(at_tmp, u_ps[:DH, :], bc_sb)
                            at_producers.append(nc.sync.dma_start(
                                out=at_sb[g // 2][DH : 2 * DH, :], in_=at_tmp
                            ))

                    # ---- phase C: O-projection for this Lq block ----
                    for lt in range(NB // P):
                        row0 = b * L + c * NB + lt * P
                        for nb in range(D // NB):
                            o_ps = psA.tile([P, NB], F32, tag="acc")
                            for t in range(2):
                                nc.tensor.matmul(
                                    o_ps,
                                    lhsT=at_sb[t][:, lt * P : (lt + 1) * P],
                                    rhs=wo_sb[t][:, nb * NB : (nb + 1) * NB],
                                    start=(t == 0),
                                    stop=(t == 1),
                                )
                            o_sb = opool.tile([P, NB], F32, tag="o")
                            nc.vector.tensor_copy(o_sb, o_ps)
                            nc.sync.dma_start(
                                out=po[row0 : row0 + P, nb * NB : (nb + 1) * NB],
                                in_=o_sb,
                            )

            # ---- reduce partials across cores; each core keeps 512 rows ----
            nc.gpsimd.collective_compute(
                "ReduceScatter",
                mybir.AluOpType.add,
                replica_groups=GROUPS,
                ins=[po[:].opt()],
                outs=[ro[:].opt()],
            )
            # int8 quantization through SBUF (NB-wide chunks reuse opool's
            # [P, NB] slot size). Pass 1 finds the per-row absmax, pass 2
            # scales by qs = QMAX/absmax and casts. The host divides by the
            # SAME qs we used here (shipped via out_s), so the reciprocal's
            # approximation error cancels exactly.
            for t in range(SH // P):
                rows = slice(t * P, (t + 1) * P)
                pm = rpool.tile([P, D // NB], F32, tag="pm")
                for nb in range(D // NB):
                    r_sb = opool.tile([P, NB], F32, tag="o")
                    nc.sync.dma_start(
                        out=r_sb, in_=ro[rows, nb * NB : (nb + 1) * NB]
                    )
                    nc.vector.tensor_reduce(
                        pm[:, nb : nb + 1], r_sb, axis=AX.X, op=AL.max,
                        apply_absolute_value=True,
                    )
                amax = rpool.tile([P, 1], F32, tag="pm")
                nc.vector.tensor_reduce(
                    amax, pm, axis=AX.X, op=AL.max, apply_absolute_value=True
                )
                qs = rpool.tile([P, 1], F32, tag="pm")
                nc.vector.reciprocal(qs, amax)
                nc.vector.tensor_scalar_mul(qs, qs, QMAX)
                nc.sync.dma_start(
                    out=out_q[rows, D : D + 4].bitcast(F32), in_=qs
                )
                for nb in range(D // NB):
                    r_sb = opool.tile([P, NB], F32, tag="o")
                    nc.sync.dma_start(
                        out=r_sb, in_=ro[rows, nb * NB : (nb + 1) * NB]
                    )
                    nc.vector.tensor_scalar_mul(r_sb, r_sb, qs)
                    # NOTE: hardware's f32->int8 tensor_copy rounds to nearest
                    # (the local simulator truncates — verified empirically:
                    # adding a +0.5*sign(x) pre-adjustment doubled the HW error
                    # while halving the sim error).
                    q_sb = opool.tile([P, NB], I8, tag="o")
                    nc.vector.tensor_copy(q_sb, r_sb)
                    nc.sync.dma_start(
                        out=out_q[rows, nb * NB : (nb + 1) * NB], in_=q_sb
                    )
    nc.compile()
    return nc


def kernel(x, Wq, Wk, Wv, Wo, trace=False):
    # Only one contiguous bf16 cast on the host; sharding is pure views
    # (run_bass_kernel_spmd's concat does the single copy). x is transposed
    # on device (XBAR transpose DMA) and AllGathered over NeuronLink.
    xb = np.asarray(x, dtype=np.float32).reshape(BL, D).astype(ml_dtypes.bfloat16)
    Wq = np.asarray(Wq, dtype=np.float32).astype(ml_dtypes.bfloat16)
    Wk = np.asarray(Wk, dtype=np.float32).astype(ml_dtypes.bfloat16)
    Wv = np.asarray(Wv, dtype=np.float32).astype(ml_dtypes.bfloat16)
    # Wo: int8 per-column quantization. The scale is shared by all cores
    # (columns are global), so partials still sum correctly on device and
    # one column-wise multiply on the host undoes it at the end.
    Wo = np.asarray(Wo, dtype=np.float32)
    wo_col = np.abs(Wo).max(axis=0) / 127.0  # [D]
    Wo_q = np.round(Wo / wo_col[None, :]).astype(np.int8)

    in_maps = []
    for i in range(NC):
        qs = slice(i * DQ, (i + 1) * DQ)
        ks = slice(i * DH, (i + 1) * DH)
        blob = np.empty(NBYTES, np.uint8)
        blob[OFF_X:OFF_WQ] = xb[i * SH : (i + 1) * SH].view(np.uint8).ravel()
        blob[OFF_WQ:OFF_WK] = np.ascontiguousarray(Wq[:, qs]).view(np.uint8).ravel()
        blob[OFF_WK:OFF_WV] = np.ascontiguousarray(Wk[:, ks]).view(np.uint8).ravel()
        blob[OFF_WV:OFF_WO] = np.ascontiguousarray(Wv[:, ks]).view(np.uint8).ravel()
        blob[OFF_WO:NBYTES] = Wo_q[qs, :].view(np.uint8).ravel()
        in_maps.append({"blob": blob})

    if "nc" not in _CACHED:
        _CACHED["nc"] = build_nc()
    nc = _CACHED["nc"]

    res = run_bass_kernel_spmd(nc, in_maps, list(range(NC)), trace=trace)
    buf = np.concatenate([r["out_q"] for r in res.results], axis=0)  # [BL, D+4] i8
    q = buf[:, :D]
    s = np.ascontiguousarray(buf[:, D : D + 4]).view(np.float32)  # [BL, 1]
    # dequant: device's own per-row scale, then Wo's per-column scale
    acc = q.astype(np.float32) / s * wo_col[None, :]
    if trace:
        kernel.last_exec_time_ns = res.exec_time_ns
        kernel.last_results = res
    return acc.reshape(B, L, D)
